# revision 122
# baseline (speedup 1.0000x reference)
"""GPT-2 attention block on 8 TRN2 NeuronCores.

Sharding (Megatron-style): core i owns heads (2i, 2i+1) for both batches.
 - QKV projection computed transposed: qkvT = Wshard^T @ X^T  -> [384, 4096]
   (rows: q0|q1|k0|k1|v0|v1 head-dim slices, cols: tokens b-major).
   X^T is DMA'd in token chunks; each chunk's projection is immediately
   followed by that chunk's attention work (b, c = divmod(n, 4)) so the
   scalar/vector/pool engines overlap the PE from the start. The last
   attention chunks are locally exp-bound, so qkv(7) and the batch-0
   recv/proj work are interleaved into them as evenly-paced per-k-tile
   fillers that keep the tensor engine fed while the exp stream catches
   up.
 - scores per (batch, head) in transposed layout S^T[k, q], causal tiles
   only, both heads side by side in one 2-bank PSUM tile; one fused exp on
   ScalarE per tile (1/sqrt(64) folded into the activation); causal mask
   applied as a post-exp 0/1 multiply on the diagonal 128x128 block, one
   head on DVE (16-bit 2x mode) and one on Pool so they run in parallel.
 - AV flipped: stationary = prob tile [128k x 128q], moving = ones-augmented
   V ([v_h | 1], 65 cols) -> psum [128 q, 65] accumulates av AND the softmax
   denominator per query partition; fully-masked (ki > q-tile) matmuls are
   skipped; AV groups are emitted as each diagonal prob tile lands so only
   the last groups wait on the chunk's final exp. reciprocal + fused
   tensor_scalar multiply normalize av in place - no transposes, no
   denominator DRAM bounce. PSUM dep tracking is bank-granular, so matmul
   groups are emitted before their readers (slot-rotated psum tiles).
 - AllToAlls reshard to sequence parallelism with p-major row order
   (row = p*nt + t~, so block j <=> partitions [16j,16j+16) and both bounce
   DMAs stay contiguous); each core runs the output projection for its
   tokens. Batch 0 in one AllToAll (overlaps batch 1); batch 1 in three
   pieces (chunks 0-1, chunk 2, chunk 3) so only the last chunk's
   bounce->collective->read chain sits in the tail; recv reads are issued
   the moment each collective output exists so they never queue behind a
   later, still-blocked bounce on the SP engine.
 - PE p-state management: dummy matmuls ramp the tensor engine to full
   clock while the first input DMAs land, bridge batch-1 chunk boundaries,
   and keep the clock ramped across the tail collective chain (an idle PE
   drops to a 2-4x slower p-state for its next 3us of work).
Output per core j: [512, 1024] bf16 (upcast host-side) - rows 0:256 =
batch0, 256:384 = batch1 first half, 384:448 / 448:512 = batch1 chunks
2 / 3; within each segment row idx decodes as (pi, t~) with
token = t~*128 + 16j + pi. Matmuls in bf16 (fp32 PSUM accumulation);
softmax in fp32. Post passes: ldweights dedup + splitting multi-wait
instructions into single-wait NoOps (this walrus build caps HW waits at 1).
"""

import numpy as np
import ml_dtypes

import concourse.bass as bass
import concourse.mybir as mybir
import concourse.tile as tile
from concourse.bass_utils import run_bass_kernel_spmd

BF16 = mybir.dt.bfloat16
F32 = mybir.dt.float32
AF = mybir.ActivationFunctionType

B, S, D, H = 2, 2048, 1024, 16
NT = B * S          # 4096 tokens, b-major
NCORES = 8
DK = D // H         # 64
SCALE = 0.125       # 1/sqrt(64)

_CACHE = {}
_NO_COLLECTIVE = False


def _build(debug_dumps=False):
    nc = bass.Bass("TRN2", target_bir_lowering=False, debug=False,
                   num_devices=NCORES)

    xT = nc.dram_tensor("xT", [D, NT], BF16, kind="ExternalInput").ap()
    wqkv = nc.dram_tensor("wqkv", [D, 384], BF16, kind="ExternalInput").ap()
    wp = nc.dram_tensor("wp", [D, D], BF16, kind="ExternalInput").ap()
    cbf16 = nc.dram_tensor("cbf16", [128, 257], BF16, kind="ExternalInput").ap()
    cf32 = nc.dram_tensor("cf32", [128, 3], F32, kind="ExternalInput").ap()
    out = nc.dram_tensor("out", [512, 1024], BF16, kind="ExternalOutput").ap()
    dbg = None
    if debug_dumps:
        dbg = {
            "dbg_qkvT": nc.dram_tensor(
                "dbg_qkvT", [128, 3, NT], BF16, kind="ExternalOutput").ap(),
            "dbg_av": nc.dram_tensor(
                "dbg_av", [128, 32, 128], BF16, kind="ExternalOutput").ap(),
            "dbg_aT": nc.dram_tensor(
                "dbg_aT", [128, 8, 512], BF16, kind="ExternalOutput").ap(),
        }

    with tile.TileContext(nc) as tc:
        _body(tc, out, xT, wqkv, wp, cbf16, cf32, dbg)
    _dedup_ldweights(nc)
    _split_multi_waits(nc)
    return nc


def _dedup_ldweights(nc):
    """Drop a back-to-back identical, wait-free Ldweights (weights already
    resident; only Matmults in between; transposes clobber -> reset)."""
    for f in nc.m.functions:
        for bb in f.blocks:
            insts = bb.instructions
            new = []
            changed = False
            last_w = None
            for inst in insts:
                nm = inst.__class__.__name__
                if getattr(inst, "engine", None) == mybir.EngineType.PE:
                    if nm == "InstLdweights":
                        si = inst.sync_info
                        key = repr(inst.ins)
                        no_waits = si is None or not si.on_wait
                        no_upd = si is None or not si.on_update
                        if key == last_w and no_waits and no_upd:
                            changed = True
                            continue  # drop duplicate load
                        last_w = key
                    elif nm == "InstMatmult":
                        if getattr(inst, "is_transpose", False):
                            last_w = None
                    else:
                        last_w = None
                new.append(inst)
            if changed:
                bb.instructions = new


def _split_multi_waits(nc):
    """Walrus caps HW sync waits at 1 per instruction: hoist extras onto
    dedicated NoOps inserted just before the offender (same engine queue)."""
    import bass_rust
    nid = [0]
    for f in nc.m.functions:
        for bb in f.blocks:
            insts = bb.instructions
            new = []
            changed = False
            for inst in insts:
                si = getattr(inst, "sync_info", None)
                if si is not None and len(si.on_wait) > 1:
                    changed = True
                    waits = list(si.on_wait)
                    for w in waits[:-1]:
                        nid[0] += 1
                        nop = mybir.InstNoOp(
                            name=f"I-waitnop-{nid[0]}", ins=[], outs=[])
                        nop.engine = inst.engine
                        nop.sync_info = bass_rust.SyncInfo(
                            on_wait=[w], on_update=[])
                        new.append(nop)
                    inst.sync_info = bass_rust.SyncInfo(
                        on_wait=[waits[-1]], on_update=list(si.on_update))
                new.append(inst)
            if changed:
                bb.instructions = new


def _body(tc, out, xT, wqkv, wp, cbf16, cf32, dbg=None):
    nc = tc.nc

    with (
        tc.tile_pool(name="persist", bufs=1) as persist,
        tc.tile_pool(name="expp", bufs=28) as expp_pool,
        tc.tile_pool(name="smalls", bufs=4) as smalls_pool,
        tc.tile_pool(name="ablk", bufs=1) as ablk_pool,
        tc.tile_pool(name="outs", bufs=3) as outs_pool,
        tc.tile_pool(name="pss", bufs=2, space="PSUM") as pss_pool,
        tc.tile_pool(name="gemm", bufs=2, space="PSUM") as gemm_pool,
        tc.tile_pool(name="avp", bufs=1, space="PSUM") as avp_pool,
        tc.tile_pool(name="tps", bufs=1, space="PSUM") as tps_pool,
        tc.tile_pool(name="dram", bufs=1, space="DRAM") as dram_pool,
    ):
        # ---- persistent SBUF ----
        xT_sb = persist.tile([128, 8, NT], BF16)        # X^T, D-tile major
        wqkv_sb = persist.tile([128, 8, 384], BF16)
        wp_sb = persist.tile([128, 8, 1024], BF16)
        qkvT_sb = persist.tile([128, 3, NT], BF16)      # q|k|v ^T rows
        v_aug = persist.tile([128, 32, 130], BF16)      # [v_h0|1|v_h1|1] per token-tile
        av_sb = persist.tile([128, 32, 128], BF16)      # normalized av, token-major
        # a^T after all-to-all, one tile per reshard region (separate
        # tiles keep dep tracking from chaining phases together)
        aT0_sb = persist.tile([128, 8, 256], BF16)      # batch 0
        aT1a_sb = persist.tile([128, 8, 128], BF16)     # batch 1 first half
        aT1bc_sb = persist.tile([128, 8, 128], BF16)    # batch 1 chunks 2|3
        cbf16_sb = persist.tile([128, 257], BF16)
        cf32_sb = persist.tile([128, 3], F32)
        ident_sb = cbf16_sb[:, 0:128]
        maskmul_sb = cbf16_sb[:, 129:257]   # [k, q]: 1.0 if q >= k else 0.0
        bqkv_sb = cf32_sb[:, 0:3]

        av_bounce = {0: dram_pool.tile([S, 128], BF16, name="avb0"),
                     "1a": dram_pool.tile([S // 2, 128], BF16, name="avb1a"),
                     "1b": dram_pool.tile([S // 4, 128], BF16, name="avb1b"),
                     "1c": dram_pool.tile([S // 4, 128], BF16, name="avb1c")}
        recv_bounce = {0: dram_pool.tile([S, 128], BF16, name="rcv0"),
                       "1a": dram_pool.tile([S // 2, 128], BF16, name="rcv1a"),
                       "1b": dram_pool.tile([S // 4, 128], BF16, name="rcv1b"),
                       "1c": dram_pool.tile([S // 4, 128], BF16, name="rcv1c")}

        # warm tile first: the PE p-state warmup matmuls depend only on it
        wtile = smalls_pool.tile([128, 64], BF16, tag="wtile")
        nc.vector.memset(wtile[:, :], 0.0)

        # ones columns of v_aug (rest overwritten by V transposes)
        nc.vector.memset(v_aug[:, :, 64:65], 1.0)
        nc.vector.memset(v_aug[:, :, 129:130], 1.0)

        # ---- ACT warmup: attach table-load pseudos to wait-free instructions
        warm = smalls_pool.tile([1, 2], F32, tag="warm")
        nc.vector.memset(warm[:, 0:1], 0.0)
        nc.scalar.activation(warm[:, 1:2], warm[:, 0:1], AF.Identity)
        nc.scalar.activation(warm[:, 1:2], warm[:, 0:1], AF.Exp)
        nc.scalar.activation(warm[:, 1:2], warm[:, 0:1], AF.Copy)

        # ---- slot-rotated persistent PSUM tiles (bank-granular pool slots
        # would otherwise blow the 8-bank budget)
        avps = avp_pool.tile([128, 7, 65], F32)     # AV psum, 7 slots
        tpss = tps_pool.tile([128, 4, 128], BF16)   # transpose psum, 4 slots
        wdum = avps[0:64, 6, 0:64]   # warm-keeping dummy target (reserved)
        av_slot = [0]
        tp_slot = [0]

        # ---- PE p-state warmup: ramp the tensor engine to full clock on
        # dummy matmuls while the first input DMAs land (ramp model: full
        # speed after 3us of continuous execution).
        wps = gemm_pool.tile([128, 512], F32, tag="gemm")
        for i in range(72):
            nc.tensor.matmul(wps[0:64, 0:64], wtile[:, 0:64], wtile[:, :],
                             start=True, stop=True)

        # ---- input DMAs. First QKV matmuls need wqkv + xT chunk 0; split
        # those by kt-pairs so early k-tiles land first.
        wqkv_r = wqkv.rearrange("(kt p) n -> p kt n", p=128)
        xT0 = xT[:, 0:512].rearrange("(kt p) w -> p kt w", p=128)
        nc.sync.dma_start(wqkv_sb[:, 0:2, :], wqkv_r[:, 0:2, :])
        nc.sync.dma_start(xT_sb[:, 0:2, 0:512], xT0[:, 0:2, :])
        nc.sync.dma_start(cf32_sb[:, :], cf32[:, :])
        nc.sync.dma_start(cbf16_sb[:, :], cbf16[:, :])
        for kt in range(2, 8, 2):
            nc.sync.dma_start(wqkv_sb[:, kt:kt + 2, :], wqkv_r[:, kt:kt + 2, :])
            nc.sync.dma_start(xT_sb[:, kt:kt + 2, 0:512], xT0[:, kt:kt + 2, :])
        for n in range(1, 8):
            nc.sync.dma_start(
                xT_sb[:, :, n * 512:(n + 1) * 512],
                xT[:, n * 512:(n + 1) * 512]
                .rearrange("(kt p) w -> p kt w", p=128))
        nc.sync.dma_start(wp_sb[:, :, :],
                          wp.rearrange("(kt p) n -> p kt n", p=128))

        def qkv_m_group(n, m):
            # qkvT[:, m, n*512:(n+1)*512] = Wshard_m^T @ X^T chunk + bias
            ps = gemm_pool.tile([128, 512], F32, tag="gemm")
            for kt in range(8):
                nc.tensor.matmul(
                    ps[:, :],
                    wqkv_sb[:, kt, m * 128:(m + 1) * 128],
                    xT_sb[:, kt, n * 512:(n + 1) * 512],
                    start=(kt == 0), stop=(kt == 7),
                )
            nc.vector.tensor_scalar_add(
                qkvT_sb[:, m, n * 512:(n + 1) * 512],
                ps[:, :], bqkv_sb[:, m:m + 1])

        def qkv_v_transposes(n):
            # V transposes -> natural layout, ones-augmented. All four
            # transposes before the copies (bank-granular PSUM deps).
            pts = []
            for t in range(n * 4, n * 4 + 4):
                sl = tp_slot[0] % 4
                tp_slot[0] += 1
                ps_t = tpss[:, sl, :]
                nc.tensor.transpose(
                    ps_t, qkvT_sb[:, 2, t * 128:(t + 1) * 128],
                    ident_sb[:, :])
                pts.append(ps_t)
            for t, ps_t in zip(range(n * 4, n * 4 + 4), pts):
                # both head blocks in one strided copy (ones col at 64 kept)
                nc.vector.tensor_copy(
                    v_aug[:, t:t + 1, 0:130]
                    .rearrange("p a (h q) -> p (a h) q", h=2)[:, :, 0:64],
                    ps_t.rearrange("p (h q) -> p h q", h=2))

        def qkv_chunk(n):
            for m in range(3):
                qkv_m_group(n, m)
            qkv_v_transposes(n)

        def attn_chunk(b, c, fillers=()):
            tok0 = b * S
            q0 = tok0 + c * 512
            nk = 4 * c + 4
            ets = []
            fillers = list(fillers)
            fstride = max(1, -(-nk // (len(fillers) + 1))) if fillers else 1
            if b == 1 and not fillers:
                # bridge the chunk-boundary exp-drain stall (keeps the PE
                # p-state ramped; these run only when PE would idle)
                for i in range(10):
                    nc.tensor.matmul(wdum, wtile[:, 0:64], wtile[:, :],
                                     start=True, stop=True)

            def av_matmuls(h, qt):
                # flipped AV: stationary = prob tile, moving = [v_h | 1];
                # psum col 64 = softmax denominator per query partition.
                gq = 4 * c + qt
                sl = av_slot[0] % 6   # slot 6 reserved for warm-keeping
                av_slot[0] += 1
                ps_av = avps[:, sl, :]
                for ki in range(gq + 1):
                    et, off = ets[ki]
                    col0 = qt * 128 - off
                    nc.tensor.matmul(
                        ps_av,
                        et[:, h, col0:col0 + 128],
                        v_aug[:, b * 16 + ki, h * 65:(h + 1) * 65],
                        start=(ki == 0), stop=(ki == gq),
                    )
                return ps_av

            def av_normalize(ps_av, h, qt):
                gq = 4 * c + qt
                rec = smalls_pool.tile([128, 1], F32, tag="rec")
                nc.vector.reciprocal(rec[:, :], ps_av[:, 64:65])
                nc.vector.tensor_scalar_mul(
                    av_sb[:, b * 16 + gq, h * 64:(h + 1) * 64],
                    ps_av[:, 0:64], rec[:, :])

            pending_norms = []

            def emit_av_matmuls(qt):
                ps0 = av_matmuls(0, qt)
                ps1 = av_matmuls(1, qt)
                pending_norms.append((ps0, ps1, qt))

            def flush_norms():
                ps0, ps1, qt = pending_norms.pop(0)
                av_normalize(ps0, 0, qt)
                av_normalize(ps1, 1, qt)

            # scores (S^T layout) + exp per k-tile; post-exp causal multiply.
            # AV group (h, qt) is emitted as soon as its last prob tile
            # (diagonal ki = 4c + qt) exists, so only the last groups wait
            # on the final exp of the chunk.
            for ki in range(nk):
                off = max(0, (ki - 4 * c)) * 128
                w = 512 - off
                ps_s = pss_pool.tile([128, 2, 512], F32, tag="pss")
                for h in range(2):
                    hp = h * 64
                    nc.tensor.matmul(
                        ps_s[:, h, :w],
                        qkvT_sb[hp:hp + 64, 1,
                                tok0 + ki * 128: tok0 + (ki + 1) * 128],
                        qkvT_sb[hp:hp + 64, 0, q0 + off: q0 + 512],
                        start=True, stop=True,
                    )
                et = expp_pool.tile([128, 2, 512], BF16, tag="expp")
                nc.scalar.activation(
                    et[:, :, :w], ps_s[:, :, :w], AF.Exp, scale=SCALE)
                if ki >= 4 * c:  # diagonal tile: zero masked (q < k) probs
                    # one head per engine so both mults run in parallel
                    nc.vector.tensor_mul(
                        et[:, 0, 0:128], et[:, 0, 0:128], maskmul_sb)
                    nc.gpsimd.tensor_mul(
                        et[:, 1, 0:128], et[:, 1, 0:128], maskmul_sb)
                ets.append((et, off))
                if ki > 4 * c:
                    # AV pair for the PREVIOUS diagonal: delaying one k-tile
                    # hides that diagonal's exp+mask latency behind this
                    # tile's scores. Normalizes lag one further pair behind:
                    # PSUM dep tracking is bank-granular, so a pair's first
                    # (start=True) write stalls on ANY pending normalize
                    # read of the shared bank - lag-2 keeps reads drained
                    # before the next pair fires (4 live slots of 6).
                    emit_av_matmuls(ki - 4 * c - 1)
                    if len(pending_norms) > 1:
                        flush_norms()
                if fillers and (ki % fstride == fstride - 1
                                or nk - 1 - ki <= len(fillers)):
                    # independent PE work keeps the tensor engine fed (and
                    # its p-state ramped) while the exp stream catches up
                    fillers.pop(0)()
            emit_av_matmuls(3)
            while pending_norms:
                flush_norms()

        def a2a(key, t0, nt):
            # all-to-all over av token-tiles [t0, t0+nt), p-major rows
            # (row = p*nt + t~): block j <=> partitions [16j, 16j+16), i.e.
            # core j receives tokens with t%128 in [16j, 16j+16). p-major
            # keeps both bounce DMAs fully contiguous per partition.
            nc.sync.dma_start(
                av_bounce[key].rearrange("(p t) d -> p t d", t=nt),
                av_sb[:, t0:t0 + nt, :])
            if _NO_COLLECTIVE:
                nc.sync.dma_start(recv_bounce[key][:, :], av_bounce[key][:, :])
            else:
                nc.gpsimd.collective_compute(
                    "AllToAll", mybir.AluOpType.bypass,
                    replica_groups=[list(range(NCORES))],
                    ins=[av_bounce[key][:, :].opt()],
                    outs=[recv_bounce[key][:, :].opt()],
                )

        a_stgs = {}

        def recv_dma(key, nt):
            # issue the recv read as soon as the collective output exists so
            # it never queues behind a later, still-blocked bounce DMA
            a_stg = ablk_pool.tile([128, nt, 128], BF16, tag=f"ablk{key}",
                                   name=f"astg{key}")
            nc.sync.dma_start(
                a_stg[:, :, :],
                recv_bounce[key].rearrange("(k p) d -> p k d", p=128))
            a_stgs[key] = a_stg

        def recv_work(key, nt, dst, col0, k0=None, kn=None):
            # rebuild a^T at aT_sb cols [col0, col0 + nt*128/spt). src block
            # = nt*16 rows; a_stg tile k spans 128 rows. (k0, kn) optionally
            # restricts to a sub-range of a_stg tiles.
            a_stg = a_stgs[key]
            rng = range(0, nt, 4) if k0 is None else range(k0, k0 + kn, 4)
            for k0 in rng:                 # groups of 4 = tpss slot count
                pts = []
                for k in range(k0, min(k0 + 4, nt)):
                    sl = tp_slot[0] % 4
                    tp_slot[0] += 1
                    ps_t = tpss[:, sl, :]
                    nc.tensor.transpose(ps_t, a_stg[:, k, :], ident_sb[:, :])
                    pts.append((k, ps_t))
                for k, ps_t in pts:
                    if nt >= 8:
                        spt = nt // 8      # a_stg tiles per src block
                        s, j = k // spt, k % spt
                        nc.vector.tensor_copy(
                            dst[:, s, col0 + j * 128: col0 + (j + 1) * 128],
                            ps_t)
                    else:                  # two 64-row src blocks per tile
                        nc.vector.tensor_copy(
                            dst[:, 2 * k:2 * k + 2, col0:col0 + 64],
                            ps_t.rearrange("p (s q) -> p s q", s=2))

        def proj_tile(r0, src, scol):
            for n0 in range(0, 1024, 512):
                ps = gemm_pool.tile([128, 512], F32, tag="gemm")
                for s in range(8):
                    nc.tensor.matmul(
                        ps[:, :],
                        src[:, s, scol:scol + 128],
                        wp_sb[:, s, n0:n0 + 512],
                        start=(s == 0), stop=(s == 7),
                    )
                o_sb = outs_pool.tile([128, 512], BF16, tag="outs")
                nc.vector.tensor_copy(o_sb[:, :], ps[:, :])
                nc.sync.dma_start(
                    out[r0:r0 + 128, n0:n0 + 512], o_sb[:, :])

        # ---- main pipeline: QKV chunk n feeds attention chunk (b, c).
        # Chunks (1,2)/(1,3) are locally exp-bound once the QKV stream runs
        # dry, so qkv(7) and the batch-0 recv/proj work are interleaved into
        # them as per-k-tile fillers. tile_wait_until stamps pin the
        # scheduler's SP-queue order for the a2a chains (its internal sim
        # otherwise reorders them and the DMA-completion counters then
        # serialize every chain into the tail).
        for n in range(6):
            qkv_chunk(n)
            b, c = divmod(n, 4)
            attn_chunk(b, c)
            if (b, c) == (0, 3):
                a2a(0, 0, 16)
                recv_dma(0, 16)
            elif (b, c) == (1, 1):
                with tc.tile_wait_until(0.100):
                    a2a("1a", 16, 8)
                    recv_dma("1a", 8)
        qkv_chunk(6)
        attn_chunk(1, 2, fillers=[
            lambda: qkv_m_group(7, 0),
            lambda: qkv_m_group(7, 1),
            lambda: qkv_m_group(7, 2),
            lambda: qkv_v_transposes(7),
        ])
        a2a("1b", 24, 4)           # chunk (1,2): chain hides under (1,3)
        recv_dma("1b", 4)
        attn_chunk(1, 3, fillers=(
            [lambda k0=k0: recv_work(0, 16, aT0_sb, 0, k0=k0, kn=4)
             for k0 in range(0, 16, 4)]
            + [lambda mt=mt: proj_tile(mt * 128, aT0_sb, mt * 128)
               for mt in (0, 1)]
        ))
        a2a("1c", 28, 4)
        recv_dma("1c", 4)
        recv_work("1a", 8, aT1a_sb, 0)
        recv_work("1b", 4, aT1bc_sb, 0)
        proj_tile(256, aT1a_sb, 0)
        # keep the PE clock ramped through the 1c bounce->collective->read
        # chain (~7us): an idle tensor engine drops to the slow p-state and
        # the tail proj would then run 2-4x slow for its first 3us. Dummy
        # matmuls target a dedicated psum slot nothing reads.
        for i in range(140):
            nc.tensor.matmul(wdum, wtile[:, 0:64], wtile[:, :],
                             start=True, stop=True)
        recv_work("1c", 4, aT1bc_sb, 64)
        proj_tile(384, aT1bc_sb, 0)

        if dbg is not None:
            nc.sync.dma_start(
                dbg["dbg_qkvT"].rearrange("p m n -> p (m n)"),
                qkvT_sb[:, :, :].rearrange("p m n -> p (m n)"))
            nc.sync.dma_start(
                dbg["dbg_av"].rearrange("p t d -> p (t d)"),
                av_sb[:, :, :].rearrange("p t d -> p (t d)"))
            nc.sync.dma_start(
                dbg["dbg_aT"][:, :, 0:256].rearrange("p s n -> p (s n)"),
                aT0_sb[:, :, :].rearrange("p s n -> p (s n)"))


def _prep_inputs(hidden_states, c_attn_w, c_attn_b, c_proj_w):
    bf16 = ml_dtypes.bfloat16
    x = np.asarray(hidden_states, dtype=np.float32).reshape(NT, D)
    xT = np.ascontiguousarray(x.T).astype(bf16)
    wp = np.ascontiguousarray(np.asarray(c_proj_w, dtype=np.float32)).astype(bf16)
    identity = np.eye(128, dtype=np.float32)
    ones = np.ones((128, 1), dtype=np.float32)
    # maskmul[k, q] (S^T diagonal tile): keep iff q >= k
    p = np.arange(128)
    maskmul = (p[None, :] >= p[:, None]).astype(np.float32)
    cbf16 = np.ascontiguousarray(
        np.concatenate([identity, ones, maskmul], axis=1)).astype(bf16)

    w = np.asarray(c_attn_w, dtype=np.float32)
    bb = np.asarray(c_attn_b, dtype=np.float32)
    in_maps = []
    for i in range(NCORES):
        cols = np.r_[i * 128:(i + 1) * 128]
        wshard = np.concatenate(
            [w[:, cols], w[:, D + cols], w[:, 2 * D + cols]], axis=1)
        bshard = np.stack(
            [bb[cols], bb[D + cols], bb[2 * D + cols]], axis=1)  # [128, 3]
        cf32 = np.ascontiguousarray(bshard).astype(np.float32)
        in_maps.append({
            "xT": xT,
            "wqkv": np.ascontiguousarray(wshard).astype(bf16),
            "wp": wp,
            "cbf16": cbf16,
            "cf32": cf32,
        })
    return in_maps


def kernel(hidden_states, c_attn_w, c_attn_b, c_proj_w, c_proj_b, _trace=False):
    if "nc" not in _CACHE:
        _CACHE["nc"] = _build()
    nc = _CACHE["nc"]
    in_maps = _prep_inputs(hidden_states, c_attn_w, c_attn_b, c_proj_w)
    try:
        res = run_bass_kernel_spmd(nc, in_maps, core_ids=list(range(NCORES)),
                                   trace=_trace)
    except (ImportError, ModuleNotFoundError):
        # NTFF profiling hook unavailable in this container
        res = run_bass_kernel_spmd(nc, in_maps, core_ids=list(range(NCORES)),
                                   trace=False)
    _CACHE["last_result"] = res
    # p-major a2a blocks: core j owns tokens with t%128 in [16j, 16j+16).
    # row idx within each segment decodes as (pi, t~): token = t~*128+16j+pi.
    idx0 = np.arange(256)
    tok0 = (idx0 % 16) * 128 + (idx0 // 16)        # b0 segment (nt=16)
    idx1 = np.arange(128)
    tok1 = (idx1 % 8) * 128 + (idx1 // 8)          # b1 first half (nt=8)
    idx2 = np.arange(64)
    tok2 = (idx2 % 4) * 128 + (idx2 // 4)          # b1 quarter segs (nt=4)
    full = np.empty((NT, D), dtype=np.float32)
    for j in range(NCORES):
        o = np.asarray(res.results[j]["out"], dtype=np.float32)
        full[tok0 + 16 * j] = o[0:256]
        full[S + tok1 + 16 * j] = o[256:384]
        full[S + 1024 + tok2 + 16 * j] = o[384:448]
        full[S + 1536 + tok2 + 16 * j] = o[448:512]
    full = full + np.asarray(c_proj_b, dtype=np.float32)[None, :]
    return full.reshape(B, S, D).astype(np.float32)


# revision 126
# speedup vs baseline: 1.0007x; 1.0007x over previous
"""GPT-2 attention block on 8 TRN2 NeuronCores.

Sharding (Megatron-style): core i owns heads (2i, 2i+1) for both batches.
 - QKV projection computed transposed: qkvT = Wshard^T @ X^T  -> [384, 4096]
   (rows: q0|q1|k0|k1|v0|v1 head-dim slices, cols: tokens b-major).
   X^T is DMA'd in token chunks; each chunk's projection is immediately
   followed by that chunk's attention work (b, c = divmod(n, 4)) so the
   scalar/vector/pool engines overlap the PE from the start. The last
   attention chunks are locally exp-bound, so qkv(7) and the batch-0
   recv/proj work are interleaved into them as evenly-paced per-k-tile
   fillers that keep the tensor engine fed while the exp stream catches
   up.
 - scores per (batch, head) in transposed layout S^T[k, q], causal tiles
   only, both heads side by side in one 2-bank PSUM tile; one fused exp on
   ScalarE per tile (1/sqrt(64) folded into the activation); causal mask
   applied as a post-exp 0/1 multiply on the diagonal 128x128 block, one
   head on DVE (16-bit 2x mode) and one on Pool so they run in parallel.
 - AV flipped: stationary = prob tile [128k x 128q], moving = ones-augmented
   V ([v_h | 1], 65 cols) -> psum [128 q, 65] accumulates av AND the softmax
   denominator per query partition; fully-masked (ki > q-tile) matmuls are
   skipped; AV groups are emitted as each diagonal prob tile lands so only
   the last groups wait on the chunk's final exp. reciprocal + fused
   tensor_scalar multiply normalize av in place - no transposes, no
   denominator DRAM bounce. PSUM dep tracking is bank-granular, so matmul
   groups are emitted before their readers (slot-rotated psum tiles).
 - AllToAlls reshard to sequence parallelism with p-major row order
   (row = p*nt + t~, so block j <=> partitions [16j,16j+16) and both bounce
   DMAs stay contiguous); each core runs the output projection for its
   tokens. Batch 0 in one AllToAll (overlaps batch 1); batch 1 in three
   pieces (chunks 0-1, chunk 2, chunk 3) so only the last chunk's
   bounce->collective->read chain sits in the tail; recv reads are issued
   the moment each collective output exists so they never queue behind a
   later, still-blocked bounce on the SP engine.
 - PE p-state management: dummy matmuls ramp the tensor engine to full
   clock while the first input DMAs land, bridge batch-1 chunk boundaries,
   and keep the clock ramped across the tail collective chain (an idle PE
   drops to a 2-4x slower p-state for its next 3us of work).
Output per core j: [512, 1024] bf16 (upcast host-side) - rows 0:256 =
batch0, 256:384 = batch1 first half, 384:448 / 448:512 = batch1 chunks
2 / 3; within each segment row idx decodes as (pi, t~) with
token = t~*128 + 16j + pi. Matmuls in bf16 (fp32 PSUM accumulation);
softmax in fp32. Post passes: ldweights dedup + splitting multi-wait
instructions into single-wait NoOps (this walrus build caps HW waits at 1).
"""

import numpy as np
import ml_dtypes

import concourse.bass as bass
import concourse.mybir as mybir
import concourse.tile as tile
from concourse.bass_utils import run_bass_kernel_spmd

BF16 = mybir.dt.bfloat16
F32 = mybir.dt.float32
AF = mybir.ActivationFunctionType

B, S, D, H = 2, 2048, 1024, 16
NT = B * S          # 4096 tokens, b-major
NCORES = 8
DK = D // H         # 64
SCALE = 0.125       # 1/sqrt(64)

_CACHE = {}
_NO_COLLECTIVE = False


def _build(debug_dumps=False):
    nc = bass.Bass("TRN2", target_bir_lowering=False, debug=False,
                   num_devices=NCORES)

    xT = nc.dram_tensor("xT", [D, NT], BF16, kind="ExternalInput").ap()
    wqkv = nc.dram_tensor("wqkv", [D, 384], BF16, kind="ExternalInput").ap()
    wp = nc.dram_tensor("wp", [D, D], BF16, kind="ExternalInput").ap()
    cbf16 = nc.dram_tensor("cbf16", [128, 257], BF16, kind="ExternalInput").ap()
    cf32 = nc.dram_tensor("cf32", [128, 3], F32, kind="ExternalInput").ap()
    out = nc.dram_tensor("out", [512, 1024], BF16, kind="ExternalOutput").ap()
    dbg = None
    if debug_dumps:
        dbg = {
            "dbg_qkvT": nc.dram_tensor(
                "dbg_qkvT", [128, 3, NT], BF16, kind="ExternalOutput").ap(),
            "dbg_av": nc.dram_tensor(
                "dbg_av", [128, 32, 128], BF16, kind="ExternalOutput").ap(),
            "dbg_aT": nc.dram_tensor(
                "dbg_aT", [128, 8, 512], BF16, kind="ExternalOutput").ap(),
        }

    with tile.TileContext(nc) as tc:
        _body(tc, out, xT, wqkv, wp, cbf16, cf32, dbg)
    _dedup_ldweights(nc)
    _split_multi_waits(nc)
    return nc


def _dedup_ldweights(nc):
    """Drop a back-to-back identical, wait-free Ldweights (weights already
    resident; only Matmults in between; transposes clobber -> reset)."""
    for f in nc.m.functions:
        for bb in f.blocks:
            insts = bb.instructions
            new = []
            changed = False
            last_w = None
            for inst in insts:
                nm = inst.__class__.__name__
                if getattr(inst, "engine", None) == mybir.EngineType.PE:
                    if nm == "InstLdweights":
                        si = inst.sync_info
                        key = repr(inst.ins)
                        no_waits = si is None or not si.on_wait
                        no_upd = si is None or not si.on_update
                        if key == last_w and no_waits and no_upd:
                            changed = True
                            continue  # drop duplicate load
                        last_w = key
                    elif nm == "InstMatmult":
                        if getattr(inst, "is_transpose", False):
                            last_w = None
                    else:
                        last_w = None
                new.append(inst)
            if changed:
                bb.instructions = new


def _split_multi_waits(nc):
    """Walrus caps HW sync waits at 1 per instruction: hoist extras onto
    dedicated NoOps inserted just before the offender (same engine queue)."""
    import bass_rust
    nid = [0]
    for f in nc.m.functions:
        for bb in f.blocks:
            insts = bb.instructions
            new = []
            changed = False
            for inst in insts:
                si = getattr(inst, "sync_info", None)
                if si is not None and len(si.on_wait) > 1:
                    changed = True
                    waits = list(si.on_wait)
                    for w in waits[:-1]:
                        nid[0] += 1
                        nop = mybir.InstNoOp(
                            name=f"I-waitnop-{nid[0]}", ins=[], outs=[])
                        nop.engine = inst.engine
                        nop.sync_info = bass_rust.SyncInfo(
                            on_wait=[w], on_update=[])
                        new.append(nop)
                    inst.sync_info = bass_rust.SyncInfo(
                        on_wait=[waits[-1]], on_update=list(si.on_update))
                new.append(inst)
            if changed:
                bb.instructions = new


def _body(tc, out, xT, wqkv, wp, cbf16, cf32, dbg=None):
    nc = tc.nc

    with (
        tc.tile_pool(name="persist", bufs=1) as persist,
        tc.tile_pool(name="expp", bufs=28) as expp_pool,
        tc.tile_pool(name="smalls", bufs=4) as smalls_pool,
        tc.tile_pool(name="ablk", bufs=1) as ablk_pool,
        tc.tile_pool(name="outs", bufs=3) as outs_pool,
        tc.tile_pool(name="pss", bufs=2, space="PSUM") as pss_pool,
        tc.tile_pool(name="gemm", bufs=2, space="PSUM") as gemm_pool,
        tc.tile_pool(name="avp", bufs=1, space="PSUM") as avp_pool,
        tc.tile_pool(name="tps", bufs=1, space="PSUM") as tps_pool,
        tc.tile_pool(name="dram", bufs=1, space="DRAM") as dram_pool,
    ):
        # ---- persistent SBUF ----
        xT_sb = persist.tile([128, 8, NT], BF16)        # X^T, D-tile major
        wqkv_sb = persist.tile([128, 8, 384], BF16)
        wp_sb = persist.tile([128, 8, 1024], BF16)
        qkvT_sb = persist.tile([128, 3, NT], BF16)      # q|k|v ^T rows
        v_aug = persist.tile([128, 32, 130], BF16)      # [v_h0|1|v_h1|1] per token-tile
        av_sb = persist.tile([128, 32, 128], BF16)      # normalized av, token-major
        # a^T after all-to-all, one tile per reshard region (separate
        # tiles keep dep tracking from chaining phases together)
        aT0_sb = persist.tile([128, 8, 256], BF16)      # batch 0
        aT1a_sb = persist.tile([128, 8, 128], BF16)     # batch 1 first half
        aT1bc_sb = persist.tile([128, 8, 128], BF16)    # batch 1 chunks 2|3
        cbf16_sb = persist.tile([128, 257], BF16)
        cf32_sb = persist.tile([128, 3], F32)
        ident_sb = cbf16_sb[:, 0:128]
        maskmul_sb = cbf16_sb[:, 129:257]   # [k, q]: 1.0 if q >= k else 0.0
        bqkv_sb = cf32_sb[:, 0:3]

        av_bounce = {0: dram_pool.tile([S, 128], BF16, name="avb0"),
                     "1a": dram_pool.tile([S // 2, 128], BF16, name="avb1a"),
                     "1b": dram_pool.tile([S // 4, 128], BF16, name="avb1b"),
                     "1c": dram_pool.tile([S // 4, 128], BF16, name="avb1c")}
        recv_bounce = {0: dram_pool.tile([S, 128], BF16, name="rcv0"),
                       "1a": dram_pool.tile([S // 2, 128], BF16, name="rcv1a"),
                       "1b": dram_pool.tile([S // 4, 128], BF16, name="rcv1b"),
                       "1c": dram_pool.tile([S // 4, 128], BF16, name="rcv1c")}

        # warm tile first: the PE p-state warmup matmuls depend only on it
        wtile = smalls_pool.tile([128, 64], BF16, tag="wtile")
        nc.vector.memset(wtile[:, :], 0.0)

        # ones columns of v_aug (rest overwritten by V transposes)
        nc.vector.memset(v_aug[:, :, 64:65], 1.0)
        nc.vector.memset(v_aug[:, :, 129:130], 1.0)

        # ---- ACT warmup: attach table-load pseudos to wait-free instructions
        warm = smalls_pool.tile([1, 2], F32, tag="warm")
        nc.vector.memset(warm[:, 0:1], 0.0)
        nc.scalar.activation(warm[:, 1:2], warm[:, 0:1], AF.Identity)
        nc.scalar.activation(warm[:, 1:2], warm[:, 0:1], AF.Exp)
        nc.scalar.activation(warm[:, 1:2], warm[:, 0:1], AF.Copy)

        # ---- slot-rotated persistent PSUM tiles (bank-granular pool slots
        # would otherwise blow the 8-bank budget)
        avps = avp_pool.tile([128, 7, 65], F32)     # AV psum, 7 slots
        tpss = tps_pool.tile([128, 4, 128], BF16)   # transpose psum, 4 slots
        wdum = avps[0:64, 6, 0:64]   # warm-keeping dummy target (reserved)
        av_slot = [0]
        tp_slot = [0]

        # ---- PE p-state warmup: ramp the tensor engine to full clock on
        # dummy matmuls while the first input DMAs land (ramp model: full
        # speed after 3us of continuous execution).
        wps = gemm_pool.tile([128, 512], F32, tag="gemm")
        for i in range(72):
            nc.tensor.matmul(wps[0:64, 0:64], wtile[:, 0:64], wtile[:, :],
                             start=True, stop=True)

        # ---- input DMAs. First QKV matmuls need wqkv + xT chunk 0; split
        # those by kt-pairs so early k-tiles land first.
        wqkv_r = wqkv.rearrange("(kt p) n -> p kt n", p=128)
        xT0 = xT[:, 0:512].rearrange("(kt p) w -> p kt w", p=128)
        nc.sync.dma_start(wqkv_sb[:, 0:2, :], wqkv_r[:, 0:2, :])
        nc.sync.dma_start(xT_sb[:, 0:2, 0:512], xT0[:, 0:2, :])
        nc.sync.dma_start(cf32_sb[:, :], cf32[:, :])
        nc.sync.dma_start(cbf16_sb[:, :], cbf16[:, :])
        for kt in range(2, 8, 2):
            nc.sync.dma_start(wqkv_sb[:, kt:kt + 2, :], wqkv_r[:, kt:kt + 2, :])
            nc.sync.dma_start(xT_sb[:, kt:kt + 2, 0:512], xT0[:, kt:kt + 2, :])
        for n in range(1, 8):
            nc.sync.dma_start(
                xT_sb[:, :, n * 512:(n + 1) * 512],
                xT[:, n * 512:(n + 1) * 512]
                .rearrange("(kt p) w -> p kt w", p=128))
        nc.sync.dma_start(wp_sb[:, :, :],
                          wp.rearrange("(kt p) n -> p kt n", p=128))

        def qkv_m_group(n, m):
            # qkvT[:, m, n*512:(n+1)*512] = Wshard_m^T @ X^T chunk + bias
            ps = gemm_pool.tile([128, 512], F32, tag="gemm")
            for kt in range(8):
                nc.tensor.matmul(
                    ps[:, :],
                    wqkv_sb[:, kt, m * 128:(m + 1) * 128],
                    xT_sb[:, kt, n * 512:(n + 1) * 512],
                    start=(kt == 0), stop=(kt == 7),
                )
            nc.vector.tensor_scalar_add(
                qkvT_sb[:, m, n * 512:(n + 1) * 512],
                ps[:, :], bqkv_sb[:, m:m + 1])

        def qkv_v_transposes(n):
            # V transposes -> natural layout, ones-augmented. All four
            # transposes before the copies (bank-granular PSUM deps).
            pts = []
            for t in range(n * 4, n * 4 + 4):
                sl = tp_slot[0] % 4
                tp_slot[0] += 1
                ps_t = tpss[:, sl, :]
                nc.tensor.transpose(
                    ps_t, qkvT_sb[:, 2, t * 128:(t + 1) * 128],
                    ident_sb[:, :])
                pts.append(ps_t)
            for t, ps_t in zip(range(n * 4, n * 4 + 4), pts):
                # both head blocks in one strided copy (ones col at 64 kept)
                nc.vector.tensor_copy(
                    v_aug[:, t:t + 1, 0:130]
                    .rearrange("p a (h q) -> p (a h) q", h=2)[:, :, 0:64],
                    ps_t.rearrange("p (h q) -> p h q", h=2))

        def qkv_chunk(n):
            for m in range(3):
                qkv_m_group(n, m)
            qkv_v_transposes(n)

        def attn_chunk(b, c, fillers=()):
            tok0 = b * S
            q0 = tok0 + c * 512
            nk = 4 * c + 4
            ets = []
            fillers = list(fillers)
            fstride = max(1, nk // (len(fillers) + 1)) if fillers else 1
            if b == 1 and not fillers:
                # bridge the chunk-boundary exp-drain stall (keeps the PE
                # p-state ramped; these run only when PE would idle)
                for i in range(10):
                    nc.tensor.matmul(wdum, wtile[:, 0:64], wtile[:, :],
                                     start=True, stop=True)

            def av_matmuls(h, qt):
                # flipped AV: stationary = prob tile, moving = [v_h | 1];
                # psum col 64 = softmax denominator per query partition.
                gq = 4 * c + qt
                sl = av_slot[0] % 6   # slot 6 reserved for warm-keeping
                av_slot[0] += 1
                ps_av = avps[:, sl, :]
                for ki in range(gq + 1):
                    et, off = ets[ki]
                    col0 = qt * 128 - off
                    nc.tensor.matmul(
                        ps_av,
                        et[:, h, col0:col0 + 128],
                        v_aug[:, b * 16 + ki, h * 65:(h + 1) * 65],
                        start=(ki == 0), stop=(ki == gq),
                    )
                return ps_av

            def av_normalize(ps_av, h, qt):
                gq = 4 * c + qt
                rec = smalls_pool.tile([128, 1], F32, tag="rec")
                nc.vector.reciprocal(rec[:, :], ps_av[:, 64:65])
                nc.vector.tensor_scalar_mul(
                    av_sb[:, b * 16 + gq, h * 64:(h + 1) * 64],
                    ps_av[:, 0:64], rec[:, :])

            pending_norms = []

            def emit_av_matmuls(qt):
                ps0 = av_matmuls(0, qt)
                ps1 = av_matmuls(1, qt)
                pending_norms.append((ps0, ps1, qt))

            def flush_norms():
                ps0, ps1, qt = pending_norms.pop(0)
                av_normalize(ps0, 0, qt)
                av_normalize(ps1, 1, qt)

            # scores (S^T layout) + exp per k-tile; post-exp causal multiply.
            # AV group (h, qt) is emitted as soon as its last prob tile
            # (diagonal ki = 4c + qt) exists, so only the last groups wait
            # on the final exp of the chunk.
            for ki in range(nk):
                off = max(0, (ki - 4 * c)) * 128
                w = 512 - off
                ps_s = pss_pool.tile([128, 2, 512], F32, tag="pss")
                for h in range(2):
                    hp = h * 64
                    nc.tensor.matmul(
                        ps_s[:, h, :w],
                        qkvT_sb[hp:hp + 64, 1,
                                tok0 + ki * 128: tok0 + (ki + 1) * 128],
                        qkvT_sb[hp:hp + 64, 0, q0 + off: q0 + 512],
                        start=True, stop=True,
                    )
                et = expp_pool.tile([128, 2, 512], BF16, tag="expp")
                nc.scalar.activation(
                    et[:, :, :w], ps_s[:, :, :w], AF.Exp, scale=SCALE)
                if ki >= 4 * c:  # diagonal tile: zero masked (q < k) probs
                    # one head per engine so both mults run in parallel
                    nc.vector.tensor_mul(
                        et[:, 0, 0:128], et[:, 0, 0:128], maskmul_sb)
                    nc.gpsimd.tensor_mul(
                        et[:, 1, 0:128], et[:, 1, 0:128], maskmul_sb)
                ets.append((et, off))
                if ki > 4 * c:
                    # AV pair for the PREVIOUS diagonal: delaying one k-tile
                    # hides that diagonal's exp+mask latency behind this
                    # tile's scores. Normalizes lag one further pair behind:
                    # PSUM dep tracking is bank-granular, so a pair's first
                    # (start=True) write stalls on ANY pending normalize
                    # read of the shared bank - lag-2 keeps reads drained
                    # before the next pair fires (4 live slots of 6).
                    emit_av_matmuls(ki - 4 * c - 1)
                    if len(pending_norms) > 1:
                        flush_norms()
                if fillers and (ki % fstride == 0
                                or nk - 1 - ki <= len(fillers)):
                    # independent PE work keeps the tensor engine fed (and
                    # its p-state ramped) while the exp stream catches up
                    fillers.pop(0)()
            emit_av_matmuls(3)
            while pending_norms:
                flush_norms()

        def a2a(key, t0, nt):
            # all-to-all over av token-tiles [t0, t0+nt), p-major rows
            # (row = p*nt + t~): block j <=> partitions [16j, 16j+16), i.e.
            # core j receives tokens with t%128 in [16j, 16j+16). p-major
            # keeps both bounce DMAs fully contiguous per partition.
            nc.sync.dma_start(
                av_bounce[key].rearrange("(p t) d -> p t d", t=nt),
                av_sb[:, t0:t0 + nt, :])
            if _NO_COLLECTIVE:
                nc.sync.dma_start(recv_bounce[key][:, :], av_bounce[key][:, :])
            else:
                nc.gpsimd.collective_compute(
                    "AllToAll", mybir.AluOpType.bypass,
                    replica_groups=[list(range(NCORES))],
                    ins=[av_bounce[key][:, :].opt()],
                    outs=[recv_bounce[key][:, :].opt()],
                )

        a_stgs = {}

        def recv_dma(key, nt):
            # issue the recv read as soon as the collective output exists so
            # it never queues behind a later, still-blocked bounce DMA
            a_stg = ablk_pool.tile([128, nt, 128], BF16, tag=f"ablk{key}",
                                   name=f"astg{key}")
            nc.sync.dma_start(
                a_stg[:, :, :],
                recv_bounce[key].rearrange("(k p) d -> p k d", p=128))
            a_stgs[key] = a_stg

        def recv_work(key, nt, dst, col0, k0=None, kn=None):
            # rebuild a^T at aT_sb cols [col0, col0 + nt*128/spt). src block
            # = nt*16 rows; a_stg tile k spans 128 rows. (k0, kn) optionally
            # restricts to a sub-range of a_stg tiles.
            a_stg = a_stgs[key]
            rng = range(0, nt, 4) if k0 is None else range(k0, k0 + kn, 4)
            for k0 in rng:                 # groups of 4 = tpss slot count
                pts = []
                for k in range(k0, min(k0 + 4, nt)):
                    sl = tp_slot[0] % 4
                    tp_slot[0] += 1
                    ps_t = tpss[:, sl, :]
                    nc.tensor.transpose(ps_t, a_stg[:, k, :], ident_sb[:, :])
                    pts.append((k, ps_t))
                for k, ps_t in pts:
                    if nt >= 8:
                        spt = nt // 8      # a_stg tiles per src block
                        s, j = k // spt, k % spt
                        nc.vector.tensor_copy(
                            dst[:, s, col0 + j * 128: col0 + (j + 1) * 128],
                            ps_t)
                    else:                  # two 64-row src blocks per tile
                        nc.vector.tensor_copy(
                            dst[:, 2 * k:2 * k + 2, col0:col0 + 64],
                            ps_t.rearrange("p (s q) -> p s q", s=2))

        def proj_tile(r0, src, scol):
            for n0 in range(0, 1024, 512):
                ps = gemm_pool.tile([128, 512], F32, tag="gemm")
                for s in range(8):
                    nc.tensor.matmul(
                        ps[:, :],
                        src[:, s, scol:scol + 128],
                        wp_sb[:, s, n0:n0 + 512],
                        start=(s == 0), stop=(s == 7),
                    )
                o_sb = outs_pool.tile([128, 512], BF16, tag="outs")
                nc.vector.tensor_copy(o_sb[:, :], ps[:, :])
                nc.sync.dma_start(
                    out[r0:r0 + 128, n0:n0 + 512], o_sb[:, :])

        # ---- main pipeline: QKV chunk n feeds attention chunk (b, c).
        # Chunks (1,2)/(1,3) are locally exp-bound once the QKV stream runs
        # dry, so qkv(7) and the batch-0 recv/proj work are interleaved into
        # them as per-k-tile fillers. tile_wait_until stamps pin the
        # scheduler's SP-queue order for the a2a chains (its internal sim
        # otherwise reorders them and the DMA-completion counters then
        # serialize every chain into the tail).
        for n in range(6):
            qkv_chunk(n)
            b, c = divmod(n, 4)
            attn_chunk(b, c)
            if (b, c) == (0, 3):
                a2a(0, 0, 16)
                recv_dma(0, 16)
            elif (b, c) == (1, 1):
                with tc.tile_wait_until(0.100):
                    a2a("1a", 16, 8)
                    recv_dma("1a", 8)
        qkv_chunk(6)
        attn_chunk(1, 2, fillers=[
            lambda: qkv_m_group(7, 0),
            lambda: qkv_m_group(7, 1),
            lambda: qkv_m_group(7, 2),
            lambda: qkv_v_transposes(7),
        ])
        a2a("1b", 24, 4)           # chunk (1,2): chain hides under (1,3)
        recv_dma("1b", 4)
        attn_chunk(1, 3, fillers=(
            [lambda k0=k0: recv_work(0, 16, aT0_sb, 0, k0=k0, kn=4)
             for k0 in range(0, 16, 4)]
            + [lambda mt=mt: proj_tile(mt * 128, aT0_sb, mt * 128)
               for mt in (0, 1)]
        ))
        a2a("1c", 28, 4)
        recv_dma("1c", 4)
        recv_work("1a", 8, aT1a_sb, 0)
        recv_work("1b", 4, aT1bc_sb, 0)
        proj_tile(256, aT1a_sb, 0)
        # keep the PE clock ramped through the 1c bounce->collective->read
        # chain (~7us): an idle tensor engine drops to the slow p-state and
        # the tail proj would then run 2-4x slow for its first 3us. Dummy
        # matmuls target a dedicated psum slot nothing reads.
        for i in range(140):
            nc.tensor.matmul(wdum, wtile[:, 0:64], wtile[:, :],
                             start=True, stop=True)
        recv_work("1c", 4, aT1bc_sb, 64)
        proj_tile(384, aT1bc_sb, 0)

        if dbg is not None:
            nc.sync.dma_start(
                dbg["dbg_qkvT"].rearrange("p m n -> p (m n)"),
                qkvT_sb[:, :, :].rearrange("p m n -> p (m n)"))
            nc.sync.dma_start(
                dbg["dbg_av"].rearrange("p t d -> p (t d)"),
                av_sb[:, :, :].rearrange("p t d -> p (t d)"))
            nc.sync.dma_start(
                dbg["dbg_aT"][:, :, 0:256].rearrange("p s n -> p (s n)"),
                aT0_sb[:, :, :].rearrange("p s n -> p (s n)"))


def _prep_inputs(hidden_states, c_attn_w, c_attn_b, c_proj_w):
    bf16 = ml_dtypes.bfloat16
    x = np.asarray(hidden_states, dtype=np.float32).reshape(NT, D)
    xT = np.ascontiguousarray(x.T).astype(bf16)
    wp = np.ascontiguousarray(np.asarray(c_proj_w, dtype=np.float32)).astype(bf16)
    identity = np.eye(128, dtype=np.float32)
    ones = np.ones((128, 1), dtype=np.float32)
    # maskmul[k, q] (S^T diagonal tile): keep iff q >= k
    p = np.arange(128)
    maskmul = (p[None, :] >= p[:, None]).astype(np.float32)
    cbf16 = np.ascontiguousarray(
        np.concatenate([identity, ones, maskmul], axis=1)).astype(bf16)

    w = np.asarray(c_attn_w, dtype=np.float32)
    bb = np.asarray(c_attn_b, dtype=np.float32)
    in_maps = []
    for i in range(NCORES):
        cols = np.r_[i * 128:(i + 1) * 128]
        wshard = np.concatenate(
            [w[:, cols], w[:, D + cols], w[:, 2 * D + cols]], axis=1)
        bshard = np.stack(
            [bb[cols], bb[D + cols], bb[2 * D + cols]], axis=1)  # [128, 3]
        cf32 = np.ascontiguousarray(bshard).astype(np.float32)
        in_maps.append({
            "xT": xT,
            "wqkv": np.ascontiguousarray(wshard).astype(bf16),
            "wp": wp,
            "cbf16": cbf16,
            "cf32": cf32,
        })
    return in_maps


def kernel(hidden_states, c_attn_w, c_attn_b, c_proj_w, c_proj_b, _trace=False):
    if "nc" not in _CACHE:
        _CACHE["nc"] = _build()
    nc = _CACHE["nc"]
    in_maps = _prep_inputs(hidden_states, c_attn_w, c_attn_b, c_proj_w)
    try:
        res = run_bass_kernel_spmd(nc, in_maps, core_ids=list(range(NCORES)),
                                   trace=_trace)
    except (ImportError, ModuleNotFoundError):
        # NTFF profiling hook unavailable in this container
        res = run_bass_kernel_spmd(nc, in_maps, core_ids=list(range(NCORES)),
                                   trace=False)
    _CACHE["last_result"] = res
    # p-major a2a blocks: core j owns tokens with t%128 in [16j, 16j+16).
    # row idx within each segment decodes as (pi, t~): token = t~*128+16j+pi.
    idx0 = np.arange(256)
    tok0 = (idx0 % 16) * 128 + (idx0 // 16)        # b0 segment (nt=16)
    idx1 = np.arange(128)
    tok1 = (idx1 % 8) * 128 + (idx1 // 8)          # b1 first half (nt=8)
    idx2 = np.arange(64)
    tok2 = (idx2 % 4) * 128 + (idx2 // 4)          # b1 quarter segs (nt=4)
    full = np.empty((NT, D), dtype=np.float32)
    for j in range(NCORES):
        o = np.asarray(res.results[j]["out"], dtype=np.float32)
        full[tok0 + 16 * j] = o[0:256]
        full[S + tok1 + 16 * j] = o[256:384]
        full[S + 1024 + tok2 + 16 * j] = o[384:448]
        full[S + 1536 + tok2 + 16 * j] = o[448:512]
    full = full + np.asarray(c_proj_b, dtype=np.float32)[None, :]
    return full.reshape(B, S, D).astype(np.float32)
